# revision 1
# baseline (speedup 1.0000x reference)
"""GCN (2-layer + mean-pool + classifier) Bass/Tile kernel for 8 Trainium2
NeuronCores, self-contained.

Sharding: dst-node partitioning (12544 nodes / 98 windows of 128 per core).
Layer 1: messages u1[src] = (x*dinv)[src] are a pure host-side layout of the
input, staged dense msg-major in DRAM; the device runs, per window, T1
accumulating PE matmuls y += S_t.T @ G_t (S_t a [128 msg, 128 dst] fp8
one-hot as the stationary operand -> fast weight load), y in PSUM [dst, CH].
Layer 2: u2 = h1*dinv is device-computed, AllGathered in two overlapped
halves, then edge-expanded with the one-hot PE gather (bucketed by
(window-group, src-block), spill via indirect row-gathers), staged through
DRAM, reloaded per window and scattered like layer 1. Pooling via per-window
one-hot matmuls into a persistent PSUM accumulator + AllReduce; classifier
on-chip. Output [512, 2] f32 (identical on every core).
"""
import numpy as np
import ml_dtypes

# ---------------------------------------------------------------- constants
N = 100000
N_PAD = 100352
NBLK = 784                     # 128-node src blocks
NCORES = 8
NPC = N_PAD // NCORES          # nodes per core = 12544
WPC = NPC // 128               # windows per core = 98
WH = WPC // 2                  # windows per allgather half = 49
NWG = 6                        # window groups of 16 (windows 0..95)
B = 8                          # bucket slots per (window, block)
NGT = NBLK * NWG               # L2 gather tiles
NST = WPC * 49                 # L2 scatter tiles (49 per window)
SPILL_CAP = [3] * 96 + [36, 36]
NSPILL = sum(SPILL_CAP)
SPILL_T0 = np.concatenate([[0], np.cumsum(SPILL_CAP)]).astype(int)
NGRAPH = 512
FP8NP = ml_dtypes.float8_e4m3
BF16NP = ml_dtypes.bfloat16

_LAST_RESULTS = None


# ------------------------------------------------------------------ patches
def _install_patches():
    import json

    import concourse.mybir as mybir
    import concourse.tile as tile_mod
    from concourse.vector_clock import ScopedClock

    if not getattr(tile_mod.TileContext, "_gcn_patched", False):
        def _drain_and_barrier(self, tick_clock, wait_clock):
            nc = self.nc
            drain_inst = nc.sync.drain()
            wait_clock.add_sem_waits(
                drain_inst.ins, ScopedClock({None: tick_clock.global_clock}))
            si = drain_inst.ins.sync_info
            waits = list(si.on_wait) if si is not None and si.on_wait else []
            if len(waits) > 1:
                si.on_wait = waits[:1]
                for w in waits[1:]:
                    extra = nc.sync.drain()
                    extra.ins.sync_info = mybir.SyncInfo(
                        on_wait=[w], on_update=[])
            nc.all_engine_barrier()
            assert self.sems is not None
            popped = nc._tile_sem_poison_stack.pop()
            assert popped is self._sem_poison
            nc.clear_and_free_semaphores(list(self.sems.allocated().values()))
            nc.all_engine_barrier()

        tile_mod.TileContext._drain_and_barrier = _drain_and_barrier
        tile_mod.TileContext._gcn_patched = True

    import concourse.bass as bass_mod

    if not getattr(bass_mod.Bass, "_wait_split_patched", False):
        orig = bass_mod.Bass.to_json_bytes

        def _split(data):
            j = json.loads(data)
            cnt = [0]

            def fix(insts):
                out = []
                for inst in insts:
                    si = inst.get("sync_info")
                    waits = si.get("on_wait") if si else None
                    if waits and len(waits) > 1:
                        for w in waits[:-1]:
                            cnt[0] += 1
                            out.append({
                                "debug": inst.get("debug", 0),
                                "engine": inst["engine"],
                                "ins": [], "outs": [],
                                "name": f"WSPL-{cnt[0]}-{inst['name']}",
                                "opcode": "EventSemaphore",
                                "sync_info": {"on_update": [], "on_wait": [w]},
                            })
                        si["on_wait"] = [waits[-1]]
                    out.append(inst)
                insts[:] = out

            def walk(d):
                if isinstance(d, dict):
                    for k, v in d.items():
                        if k == "instructions" and isinstance(v, list):
                            fix(v)
                        else:
                            walk(v)
                elif isinstance(d, list):
                    for e in d:
                        walk(e)

            walk(j)
            return json.dumps(j).encode()

        def to_json_bytes(self, *a, **kw):
            return _split(orig(self, *a, **kw))

        bass_mod.Bass.to_json_bytes = to_json_bytes
        bass_mod.Bass._wait_split_patched = True


# ----------------------------------------------------------------- cpu prep
def _prepare(x, edge_index, batch, W1, b1, W2, b2, Wc, bc):
    src = np.asarray(edge_index[0], dtype=np.int64)
    dst = np.asarray(edge_index[1], dtype=np.int64)
    batch = np.asarray(batch, dtype=np.int64)
    x = np.asarray(x, dtype=np.float32)

    deg = np.ones(N_PAD, dtype=np.float32)
    np.add.at(deg, dst, 1.0)
    dinv = (1.0 / np.sqrt(deg)).astype(np.float32)

    u1_rows = np.zeros((N_PAD, 8), dtype=np.float32)
    u1_rows[:N] = x
    u1_rows *= dinv[:, None]
    u1_rows = u1_rows.astype(BF16NP)

    # edge list incl. self-loops, sorted by dst
    loop = np.arange(N_PAD, dtype=np.int64)
    s_all = np.concatenate([src, loop])
    d_all = np.concatenate([dst, loop])
    order = np.argsort(d_all, kind="stable")
    s_all, d_all = s_all[order], d_all[order]

    win = (d_all // 128).astype(np.int64)
    wcnt = np.bincount(win, minlength=N_PAD // 128)
    T1 = int(np.ceil(wcnt.max() / 128))
    wstart = np.concatenate([[0], np.cumsum(wcnt)])

    eye_fp8 = np.eye(128, dtype=np.float32).astype(FP8NP)
    eye_bf16 = np.eye(128, dtype=np.float32).astype(BF16NP)
    eye_f32 = np.eye(128, dtype=np.float32)

    cnt = np.zeros(NGRAPH, dtype=np.float32)
    np.add.at(cnt, batch, 1.0)
    cnt_inv = np.where(cnt > 0, 1.0 / np.maximum(cnt, 1.0), 1.0).astype(np.float32)
    cnt_inv = np.ascontiguousarray(cnt_inv.reshape(4, 128).T)[:, :, None].copy()

    per_core = []
    for c in range(NCORES):
        # ---- layer 1: host-staged dense messages + scatter one-hots
        st1 = np.zeros((128, WPC, T1, 8), dtype=BF16NP)
        s_dst1 = np.zeros((128, WPC * T1, 128), dtype=FP8NP)
        for w in range(WPC):
            gw = c * WPC + w
            lo, hi = int(wstart[gw]), int(wstart[gw + 1])
            n = hi - lo
            k = np.arange(n)
            t = k // 128
            q = k % 128
            st1[q, w, t, :] = u1_rows[s_all[lo:hi]]
            s_dst1[q, w * T1 + t, (d_all[lo:hi] % 128)] = 1.0

        # ---- layer 2: bucketed one-hot gather structures (baseline scheme)
        lo, hi = int(wstart[c * WPC]), int(wstart[(c + 1) * WPC])
        es, ed = s_all[lo:hi], d_all[lo:hi]
        w_ = (ed // 128) - c * WPC
        I = es // 128
        rs = es % 128
        rd = ed % 128

        o2 = np.lexsort((I, w_))
        w2, I2, rs2, rd2 = w_[o2], I[o2], rs[o2], rd[o2]
        key = w2 * NBLK + I2
        _, start, cnts = np.unique(key, return_index=True, return_counts=True)
        rank = np.arange(len(key)) - np.repeat(start, cnts)

        main = (w2 < 96) & (rank < B)
        spm = ~main

        wm, Im, rsm, rdm, bm = (a[main] for a in (w2, I2, rs2, rd2, rank))
        g = wm // 16
        w_lo = wm % 16
        I_lo, I_hi = Im % 16, Im // 16
        gt = g * NBLK + (I_lo * 49 + I_hi)
        slot = w_lo * 8 + bm
        s_src = np.zeros((128, NGT, 128), dtype=FP8NP)
        s_src[rsm, gt, slot] = 1.0
        st = wm * 49 + I_hi
        sp_part = bm * 16 + I_lo
        s_dst2 = np.zeros((128, NST, 128), dtype=FP8NP)
        s_dst2[sp_part, st, rdm] = 1.0

        ws, Is, rss, rds = (a[spm] for a in (w2, I2, rs2, rd2))
        o3 = np.argsort(ws, kind="stable")
        ws, Is, rss, rds = ws[o3], Is[o3], rss[o3], rds[o3]
        wstart2 = np.searchsorted(ws, np.arange(WPC + 1))
        spill_idx = np.zeros((128, NSPILL), dtype=np.int32)
        s_spill = np.zeros((128, NSPILL, 128), dtype=FP8NP)
        for wi in range(WPC):
            a, bnd = wstart2[wi], wstart2[wi + 1]
            nsp = bnd - a
            if nsp > SPILL_CAP[wi] * 128:
                raise RuntimeError(
                    f"core {c} window {wi}: spill {nsp} > {SPILL_CAP[wi]*128}")
            k = np.arange(nsp)
            t = SPILL_T0[wi] + k // 128
            p = k % 128
            spill_idx[p, t] = (rss[a:bnd] * NBLK + Is[a:bnd]).astype(np.int32)
            s_spill[p, t, rds[a:bnd]] = 1.0

        base = c * NPC
        dinv_col = np.ascontiguousarray(
            dinv[base:base + NPC].reshape(WPC, 128).T).copy()  # [128, WPC]

        s_pool = np.zeros((128, WPC * 4, 128), dtype=FP8NP)
        nodes = np.arange(base, base + NPC)
        valid = nodes < N
        gids = batch[np.minimum(nodes, N - 1)]
        wv = (nodes - base) // 128
        pv = (nodes - base) % 128
        s_pool[pv[valid], wv[valid] * 4 + (gids[valid] // 128),
               gids[valid] % 128] = 1.0

        per_core.append({
            "st1": st1, "s_dst1": s_dst1,
            "s_src": s_src, "s_dst2": s_dst2,
            "s_spilldst": s_spill, "spill_idx": spill_idx,
            "dinv_col": dinv_col, "s_pool": s_pool,
            "w1": np.asarray(W1, np.float32).astype(BF16NP),
            "b1": np.asarray(b1, np.float32).reshape(32, 1).copy(),
            "w2": np.asarray(W2, np.float32).astype(BF16NP),
            "b2": np.asarray(b2, np.float32).reshape(32, 1).copy(),
            "wc": np.asarray(Wc, np.float32).copy(),
            "bc": np.asarray(bc, np.float32).reshape(2, 1).copy(),
            "eye_fp8": eye_fp8, "eye_bf16": eye_bf16, "eye_f32": eye_f32,
            "cnt_inv": cnt_inv,
        })
    return per_core, T1


# ------------------------------------------------------------------ builder
def _build_nc(T1):
    import concourse.bass as bass
    import concourse.mybir as mybir
    from concourse.tile import TileContext

    FP8 = mybir.dt.float8e4
    BF16 = mybir.dt.bfloat16
    F32 = mybir.dt.float32
    I32 = mybir.dt.int32
    AF = mybir.ActivationFunctionType

    nc = bass.Bass(target_bir_lowering=True)

    def inp(name, shape, dt):
        return nc.dram_tensor(name, shape, dt, kind="ExternalInput")

    st1 = inp("st1", [128, WPC, T1, 8], BF16)
    s_dst1 = inp("s_dst1", [128, WPC * T1, 128], FP8)
    s_src = inp("s_src", [128, NGT, 128], FP8)
    s_dst2 = inp("s_dst2", [128, NST, 128], FP8)
    s_spill = inp("s_spilldst", [128, NSPILL, 128], FP8)
    spill_idx = inp("spill_idx", [128, NSPILL], I32)
    dinv_col = inp("dinv_col", [128, WPC], F32)
    s_pool = inp("s_pool", [128, WPC * 4, 128], FP8)
    w1 = inp("w1", [8, 32], BF16)
    b1 = inp("b1", [32, 1], F32)
    w2 = inp("w2", [32, 32], BF16)
    b2 = inp("b2", [32, 1], F32)
    wc = inp("wc", [32, 2], F32)
    bc = inp("bc", [2, 1], F32)
    eye_fp8 = inp("eye_fp8", [128, 128], FP8)
    eye_bf16 = inp("eye_bf16", [128, 128], BF16)
    eye_f32 = inp("eye_f32", [128, 128], F32)
    cnt_inv = inp("cnt_inv", [128, 4, 1], F32)
    out = nc.dram_tensor("out", [NGRAPH, 2], F32, kind="ExternalOutput")

    u2_loc = [nc.dram_tensor(f"u2_loc{h}", [WH * 128, 32], BF16)
              for h in range(2)]
    u2_gh = [nc.dram_tensor(f"u2_gh{h}", [NCORES, WH * 128, 32], BF16)
             for h in range(2)]
    u2_dram = nc.dram_tensor("u2_dram", [128 * NBLK, 32], BF16)
    g1_dram = nc.dram_tensor("g1_dram", [128, NBLK, 32], BF16)
    pool_in = nc.dram_tensor("pool_in", [NGRAPH, 32], F32)
    pool_out = nc.dram_tensor("pool_out", [NGRAPH, 32], F32)

    groups = [list(range(NCORES))]
    CHUNKS = [(s, min(4, WPC - s)) for s in range(0, WPC, 4)]

    with TileContext(nc) as tc:
        with tc.tile_pool(name="glob", bufs=1) as gl, \
             tc.tile_pool(name="yps", bufs=2, space="PSUM") as yp, \
             tc.tile_pool(name="trps", bufs=1, space="PSUM") as tp, \
             tc.tile_pool(name="hps", bufs=2, space="PSUM") as hp, \
             tc.tile_pool(name="gbps", bufs=2, space="PSUM") as gbp:

            def load(t, src_ap):
                nc.sync.dma_start(out=t[:], in_=src_ap[:])
                return t

            dinv_t = load(gl.tile([128, WPC], F32, tag="dinvc", name="dinvc"),
                          dinv_col)
            eye8_t = load(gl.tile([128, 128], FP8, tag="eye8", name="eye8"),
                          eye_fp8)
            eyeb_t = load(gl.tile([128, 128], BF16, tag="eyeb", name="eyeb"),
                          eye_bf16)
            eyef_t = load(gl.tile([128, 128], F32, tag="eyef", name="eyef"),
                          eye_f32)
            w1_t = load(gl.tile([8, 32], BF16, tag="w1t", name="w1t"), w1)
            b1_t = load(gl.tile([32, 1], F32, tag="b1t", name="b1t"), b1)
            w2_t = load(gl.tile([32, 32], BF16, tag="w2t", name="w2t"), w2)
            b2_t = load(gl.tile([32, 1], F32, tag="b2t", name="b2t"), b2)
            wc_t = load(gl.tile([32, 2], F32, tag="wct", name="wct"), wc)
            bc_t = load(gl.tile([2, 1], F32, tag="bct", name="bct"), bc)
            cnt_t = load(gl.tile([128, 4, 1], F32, tag="cntt", name="cntt"),
                         cnt_inv)
            spidx_t = load(gl.tile([128, NSPILL], I32, tag="spidx",
                                   name="spidx"), spill_idx)

            spool_t = load(gl.tile([128, WPC * 4, 128], FP8, tag="spoolt",
                                   name="spoolt"), s_pool)

            u2_self = [gl.tile([128, WH, 32], BF16, tag=f"u2s{h}",
                               name=f"u2s{h}") for h in range(2)]
            u2_t = gl.tile([128, NBLK, 32], BF16, tag="u2t", name="u2t")
            pool_acc = gl.tile([128, 4, 32], F32, tag="poolacc",
                               name="poolacc")
            nc.vector.memset(pool_acc[:], 0.0)

            def scatter_win(sd, sj0, Gv, nmm, y_ps, first, stop_last):
                for t in range(nmm):
                    nc.tensor.matmul(
                        out=y_ps[:], lhsT=sd[:, sj0 + t, :],
                        rhs=Gv[:, t, :],
                        start=(first and t == 0),
                        stop=(stop_last and t == nmm - 1),
                        skip_group_check=True)

            # ---------------- layer 1 (host-staged messages)
            with tc.tile_pool(name="L1", bufs=2) as lp1, \
                 tc.tile_pool(name="L1g", bufs=3) as gp1, \
                 tc.tile_pool(name="L1s", bufs=4) as sp1:

                def fin1(w, y_ps, lp):
                    z1r = lp.tile([128, 8], BF16, tag="z1r")
                    nc.vector.tensor_scalar(
                        out=z1r[:], in0=y_ps[:],
                        scalar1=dinv_t[:, w:w + 1], scalar2=None,
                        op0=mybir.AluOpType.mult)
                    z1T_ps = tp.tile([8, 128], BF16, tag="trp")
                    nc.tensor.transpose(out=z1T_ps[:], in_=z1r[:],
                                        identity=eyeb_t[:])
                    z1T = lp.tile([8, 128], BF16, tag="z1T")
                    nc.vector.tensor_copy(out=z1T[:], in_=z1T_ps[:])
                    h1ps = hp.tile([32, 128], F32, tag="hps")
                    nc.tensor.matmul(out=h1ps[:], lhsT=w1_t[:], rhs=z1T[:],
                                     start=True, stop=True,
                                     skip_group_check=True)
                    h1T = lp.tile([32, 128], BF16, tag="h1T")
                    nc.scalar.activation(out=h1T[:], in_=h1ps[:],
                                         func=AF.Relu, bias=b1_t[:],
                                         scale=1.0)
                    u2T_ps = tp.tile([128, 32], BF16, tag="trp2")
                    nc.tensor.transpose(out=u2T_ps[:], in_=h1T[:],
                                        identity=eyeb_t[:32, :32])
                    h = w // WH
                    nc.vector.tensor_scalar(
                        out=u2_self[h][:, w - h * WH, :], in0=u2T_ps[:],
                        scalar1=dinv_t[:, w:w + 1], scalar2=None,
                        op0=mybir.AluOpType.mult)

                for (w0, nw) in CHUNKS:
                    G = gp1.tile([128, 4, T1, 8], BF16, tag="g1")
                    nc.sync.dma_start(
                        out=G[:, :nw, :, :],
                        in_=st1[:, w0:w0 + nw, :, :])
                    for wi in range(nw):
                        w = w0 + wi
                        sd = sp1.tile([128, T1, 128], FP8, tag="sd1")
                        eng = nc.sync if w % 2 == 0 else nc.scalar
                        eng.dma_start(
                            out=sd[:],
                            in_=s_dst1[:, w * T1:(w + 1) * T1, :])
                        y_ps = yp.tile([128, 8], F32, tag="yps")
                        scatter_win(sd, 0, G[:, wi], T1, y_ps, True, True)
                        fin1(w, y_ps, lp1)

                # allgather halves
                for h in range(2):
                    nc.sync.dma_start(
                        out=u2_loc[h][:].rearrange("(w p) c -> p w c", p=128),
                        in_=u2_self[h][:])
                    nc.gpsimd.collective_compute(
                        "AllGather", mybir.AluOpType.bypass,
                        replica_groups=groups,
                        ins=[u2_loc[h].ap().opt()],
                        outs=[u2_gh[h].ap().opt()])

            # assemble u2 table (SBUF, block-major) + u2_dram (for spills)
            for h in range(2):
                for cc in range(NCORES):
                    nc.sync.dma_start(
                        out=u2_t[:, cc * WPC + h * WH:
                                 cc * WPC + h * WH + WH, :],
                        in_=u2_gh[h][cc].rearrange("(w p) c -> p w c", p=128))
            nc.sync.dma_start(
                out=u2_dram[:].rearrange("(p b) c -> p b c", p=128),
                in_=u2_t[:])

            # ---------------- layer 2
            with tc.tile_pool(name="L2", bufs=2) as lp2, \
                 tc.tile_pool(name="L2g", bufs=4) as g2p, \
                 tc.tile_pool(name="L2s", bufs=4) as sp2, \
                 tc.tile_pool(name="L2sp", bufs=1) as spp:

                # spill row-gathers (indirect)
                spillG = spp.tile([128, NSPILL, 32], BF16)
                for t in range(NSPILL):
                    nc.gpsimd.indirect_dma_start(
                        out=spillG[:, t, :], out_offset=None,
                        in_=u2_dram[:],
                        in_offset=bass.IndirectOffsetOnAxis(
                            ap=spidx_t[:, t:t + 1], axis=0))

                # gather phase: one-hot matmuls -> g1_dram staging
                for g in range(NWG):
                    for ch0 in range(0, NBLK, 16):
                        nb = min(16, NBLK - ch0)
                        ssrc = lp2.tile([128, 16, 128], FP8, tag="ssrc")
                        eng = nc.sync if (ch0 // 16) % 2 == 0 else nc.scalar
                        eng.dma_start(
                            out=ssrc[:, :nb, :],
                            in_=s_src[:, g * NBLK + ch0:
                                      g * NBLK + ch0 + nb, :])
                        gbank = gbp.tile([128, 16, 32], F32, tag="gbank")
                        for i in range(nb):
                            Ip = ch0 + i
                            I = (Ip % 49) * 16 + Ip // 49
                            nc.tensor.matmul(
                                out=gbank[:, i, :], lhsT=ssrc[:, i, :],
                                rhs=u2_t[:, I, :], start=True, stop=True,
                                skip_group_check=True)
                        stag = lp2.tile([128, 16, 32], BF16, tag="stag")
                        nc.vector.tensor_copy(out=stag[:, :nb, :],
                                              in_=gbank[:, :nb, :])
                        engw = nc.scalar if (ch0 // 16) % 2 == 0 else nc.sync
                        engw.dma_start(
                            out=g1_dram[:, ch0:ch0 + nb, :],
                            in_=stag[:, :nb, :])

                    def spill_win(w, y_ps, first):
                        t0, t1 = int(SPILL_T0[w]), int(SPILL_T0[w + 1])
                        ssp = lp2.tile([128, 36, 128], FP8, tag="sspill")
                        enge = nc.sync if w % 2 == 0 else nc.scalar
                        enge.dma_start(out=ssp[:, :t1 - t0, :],
                                       in_=s_spill[:, t0:t1, :])
                        for k in range(t1 - t0):
                            nc.tensor.matmul(
                                out=y_ps[:], lhsT=ssp[:, k, :],
                                rhs=spillG[:, t0 + k, :],
                                start=(first and k == 0),
                                stop=(k == t1 - t0 - 1),
                                skip_group_check=True)

                    def fin2(w, y_ps, lp):
                        z2r = lp.tile([128, 32], BF16, tag="z2r")
                        nc.vector.tensor_scalar(
                            out=z2r[:], in0=y_ps[:],
                            scalar1=dinv_t[:, w:w + 1], scalar2=None,
                            op0=mybir.AluOpType.mult)
                        z2T_ps = tp.tile([32, 128], BF16, tag="trp")
                        nc.tensor.transpose(out=z2T_ps[:], in_=z2r[:],
                                            identity=eyeb_t[:])
                        z2T = lp.tile([32, 128], BF16, tag="z2T")
                        nc.vector.tensor_copy(out=z2T[:], in_=z2T_ps[:])
                        h2ps = hp.tile([32, 128], F32, tag="hps")
                        nc.tensor.matmul(out=h2ps[:], lhsT=w2_t[:],
                                         rhs=z2T[:], start=True, stop=True,
                                         skip_group_check=True)
                        h2T = lp.tile([32, 128], BF16, tag="h2T")
                        nc.scalar.activation(out=h2T[:], in_=h2ps[:],
                                             func=AF.Relu, bias=b2_t[:],
                                             scale=1.0)
                        h2r_ps = tp.tile([128, 32], BF16, tag="trp2")
                        nc.tensor.transpose(out=h2r_ps[:], in_=h2T[:],
                                            identity=eyeb_t[:32, :32])
                        h2row = lp.tile([128, 32], BF16, tag="h2row")
                        nc.vector.tensor_copy(out=h2row[:], in_=h2r_ps[:])
                        pool_ps = hp.tile([128, 4, 32], F32, tag="hps")
                        for gg in range(4):
                            nc.tensor.matmul(
                                out=pool_ps[:, gg, :],
                                lhsT=spool_t[:, w * 4 + gg, :], rhs=h2row[:],
                                start=True, stop=True,
                                skip_group_check=True)
                        nc.vector.tensor_tensor(
                            out=pool_acc[:], in0=pool_acc[:], in1=pool_ps[:],
                            op=mybir.AluOpType.add)

                    for w_lo in range(16):
                        w = g * 16 + w_lo
                        G2 = g2p.tile([128, 49, 32], BF16, tag="g2")
                        for bb in range(8):
                            eng2 = nc.sync if bb % 2 == 0 else nc.scalar
                            eng2.dma_start(
                                out=G2[16 * bb:16 * (bb + 1), :, :],
                                in_=g1_dram[8 * w_lo + bb, :, :].rearrange(
                                    "(il ih) c -> il ih c", il=16))
                        sd2 = sp2.tile([128, 49, 128], FP8, tag="sd2")
                        eng = nc.sync if w % 2 == 0 else nc.scalar
                        eng.dma_start(
                            out=sd2[:],
                            in_=s_dst2[:, w * 49:(w + 1) * 49, :])
                        y_ps = yp.tile([128, 32], F32, tag="yps")
                        scatter_win(sd2, 0, G2, 49, y_ps, True, False)
                        spill_win(w, y_ps, False)
                        fin2(w, y_ps, lp2)
                for w in (96, 97):
                    y_ps = yp.tile([128, 32], F32, tag="yps")
                    spill_win(w, y_ps, True)
                    fin2(w, y_ps, lp2)

            # ---------------- head
            with tc.tile_pool(name="head", bufs=1) as hd:
                for g in range(4):
                    nc.sync.dma_start(
                        out=pool_in[g * 128:(g + 1) * 128, :],
                        in_=pool_acc[:, g, :])
                nc.gpsimd.collective_compute(
                    "AllReduce", mybir.AluOpType.add, replica_groups=groups,
                    ins=[pool_in.ap().opt()], outs=[pool_out.ap().opt()])
                mean = hd.tile([128, 4, 32], F32)
                for g in range(4):
                    p2 = hd.tile([128, 32], F32, tag="p2")
                    nc.sync.dma_start(
                        out=p2[:],
                        in_=pool_out[g * 128:(g + 1) * 128, :])
                    nc.vector.tensor_scalar(
                        out=mean[:, g, :], in0=p2[:],
                        scalar1=cnt_t[:, g, :], scalar2=None,
                        op0=mybir.AluOpType.mult)
                for g in range(4):
                    trf = tp.tile([32, 128], F32, tag="trp")
                    nc.tensor.transpose(out=trf[:], in_=mean[:, g, :],
                                        identity=eyef_t[:])
                    trsb = hd.tile([32, 128], F32, tag="trsb")
                    nc.vector.tensor_copy(out=trsb[:], in_=trf[:])
                    ops = hp.tile([2, 128], F32, tag="hps")
                    nc.tensor.matmul(out=ops[:], lhsT=wc_t[:], rhs=trsb[:],
                                     start=True, stop=True,
                                     skip_group_check=True)
                    res = hd.tile([2, 128], F32, tag="res")
                    nc.vector.tensor_scalar(
                        out=res[:], in0=ops[:], scalar1=bc_t[:],
                        scalar2=None, op0=mybir.AluOpType.add)
                    for k in range(2):
                        nc.sync.dma_start(
                            out=out[g * 128:(g + 1) * 128, k:k + 1],
                            in_=res[k:k + 1, :])
    return nc


# ------------------------------------------------------------------- runner
def kernel(**inputs):
    global _LAST_RESULTS
    import os

    _install_patches()
    from concourse.bass_utils import run_bass_kernel_spmd

    per_core, T1 = _prepare(**inputs)
    nc = _build_nc(T1)
    trace = bool(os.environ.get("GCN_TRACE"))
    kw = {}
    if trace:
        kw = dict(trace=True, trace_cores=[0, 3])
    res = run_bass_kernel_spmd(
        nc, per_core, core_ids=list(range(NCORES)), **kw)
    _LAST_RESULTS = res
    return np.asarray(res.results[0]["out"], dtype=np.float32)

